# revision 1
# baseline (speedup 1.0000x reference)
"""Bass/Trainium2 kernel for nn_CrossAttention_57964878627478.

Reference computation (per batch b, per direction):
    q = Wq @ src + bq            [32, 4096]   (src = x for dir 0, y for dir 1)
    k = Wk @ ctx + bk            [32, 4096]   (ctx = the other tensor)
    v = Wv @ ctx + bv            [256, 4096]
    attn = softmax_j(q^T k)      [4096, 4096]
    out  = v @ attn^T            [256, 4096]

Sharding: 8 independent (batch, direction) pairs -> one per NeuronCore.

Per-core kernel layout choices:
  * S^T = k^T q computed in [j, i] layout directly (no transposes needed
    anywhere): lhsT = k strip [K=32, M=128 j], rhs = q strip [K=32, N=512 i].
    K=32 matmuls are packed 4-way with PE row tiling (tile_position) using
    4x-replicated q/k (replication is free: host tiles Wq^T/Wk^T columns).
  * exp on ScalarE, PSUM->SBUF, bf16 out, constant bias shift (softmax is
    shift-invariant; global max score ~34 fits fp32/bf16 range comfortably).
  * out^T[i, c] = P^T.T @ v^T via lhsT = P^T tile, rhs = v^T. An extra ones
    column appended to v^T makes column 256 of the PSUM accumulator the
    softmax denominator (free). Normalization is then a native per-partition
    tensor_scalar multiply. v carries its bias (folded in via a K=1 matmul
    with a ones lhsT), so out = psum[:, :256] * (1/psum[:, 256]) exactly.
  * q/k in fp16 (3 extra mantissa bits vs bf16 -> ~4x better end-to-end
    error), P/v in bf16 (P needs bf16's exponent range: fp16 underflows).
  * Output written as out^T [4096, 256] fp32; host transposes.
"""

import sys

if "/opt/trn_rl_repo" not in sys.path:
    sys.path.insert(0, "/opt/trn_rl_repo")

import numpy as np
import ml_dtypes

C = 256
CQ = 32
HW = 4096
B = 4
N_CORES = 8
EXP_BIAS = -12.0

_cache = {}


def _build_program():
    from contextlib import ExitStack

    import concourse.bacc as bacc
    import concourse.mybir as mybir
    import concourse.tile as tile

    fp16 = mybir.dt.float16
    bf16 = mybir.dt.bfloat16
    f32 = mybir.dt.float32

    nc = bacc.Bacc(None, target_bir_lowering=False, debug=False)
    SRC = nc.dram_tensor("src", [C, HW], fp16, kind="ExternalInput")
    CTX = nc.dram_tensor("ctx", [C, HW], fp16, kind="ExternalInput")
    WQT = nc.dram_tensor("wqt", [C, 128], fp16, kind="ExternalInput")
    WKT = nc.dram_tensor("wkt", [C, 128], fp16, kind="ExternalInput")
    WVT = nc.dram_tensor("wvt", [C, C], fp16, kind="ExternalInput")
    BQ = nc.dram_tensor("bq_rep", [128, 1], f32, kind="ExternalInput")
    BK = nc.dram_tensor("bk_rep", [128, 1], f32, kind="ExternalInput")
    BV = nc.dram_tensor("bv_row", [1, C], f32, kind="ExternalInput")
    OUT = nc.dram_tensor("out_t", [HW, C], f32, kind="ExternalOutput")

    Exp = mybir.ActivationFunctionType.Exp

    import os
    ST_GROUP = int(os.environ.get("K_ST_GROUP", "2"))   # j chunks per S^T psum tile
    ST_BUFS = int(os.environ.get("K_ST_BUFS", "3"))
    N_GROUPS = 32 // ST_GROUP                           # S^T groups per i tile
    P_BUFS = 2 * N_GROUPS

    with tile.TileContext(nc) as tc, ExitStack() as ctx:
        consts = ctx.enter_context(tc.tile_pool(name="consts", bufs=1))
        # PSUM budget (8 banks): st ST_BUFS x [128, 512*ST_GROUP] (also used
        # by phase-0 projections) + out 2x[128,257]=2
        ps_st = ctx.enter_context(tc.tile_pool(name="ps_st", bufs=ST_BUFS, space="PSUM"))
        ps_out = ctx.enter_context(tc.tile_pool(name="ps_out", bufs=2, space="PSUM"))
        p_pool = ctx.enter_context(tc.tile_pool(name="p_pool", bufs=P_BUFS))
        o_pool = ctx.enter_context(tc.tile_pool(name="o_pool", bufs=4))
        r_pool = ctx.enter_context(tc.tile_pool(name="r_pool", bufs=4))

        # ---- constant / input staging ----
        src_sb = consts.tile([128, 2, HW], fp16, tag="src_sb")
        ctx_sb = consts.tile([128, 2, HW], fp16, tag="ctx_sb")
        src_r = SRC[:].rearrange("(c p) j -> p c j", p=128)
        ctx_r = CTX[:].rearrange("(c p) j -> p c j", p=128)
        for n in range(8):
            ns = slice(n * 512, (n + 1) * 512)
            nc.sync.dma_start(out=src_sb[:, :, ns], in_=src_r[:, :, ns])
            nc.sync.dma_start(out=ctx_sb[:, :, ns], in_=ctx_r[:, :, ns])

        wqt_sb = consts.tile([128, 2, 128], fp16, tag="wqt_sb")
        wkt_sb = consts.tile([128, 2, 128], fp16, tag="wkt_sb")
        wvt_sb = consts.tile([128, 2, C], fp16, tag="wvt_sb")
        nc.sync.dma_start(out=wqt_sb, in_=WQT[:].rearrange("(c p) m -> p c m", p=128))
        nc.sync.dma_start(out=wkt_sb, in_=WKT[:].rearrange("(c p) m -> p c m", p=128))
        nc.sync.dma_start(out=wvt_sb, in_=WVT[:].rearrange("(c p) m -> p c m", p=128))
        bq_sb = consts.tile([128, 1], f32, tag="bq_sb")
        bk_sb = consts.tile([128, 1], f32, tag="bk_sb")
        bv_bc = consts.tile([128, C], f32, tag="bv_bc")
        nc.sync.dma_start(out=bq_sb, in_=BQ[:])
        nc.sync.dma_start(out=bk_sb, in_=BK[:])
        nc.sync.dma_start(out=bv_bc, in_=BV[:].to_broadcast((128, C)))
        ebias_sb = consts.tile([128, 1], f32, tag="ebias_sb")
        nc.vector.memset(ebias_sb, EXP_BIAS)

        q_sb = consts.tile([128, HW], fp16, tag="q_sb")
        k_sb = consts.tile([128, HW], fp16, tag="k_sb")
        vT_sb = consts.tile([128, 32, 257], bf16, tag="vT_sb")
        nc.vector.memset(vT_sb[:, :, 256], 1.0)

        # ---- projections ----
        # q_rep/k_rep [128, HW]: Wq^T tiled 4x along columns by the host, so
        # the 4 partition strips hold identical copies of q (for row tiling).
        for n in range(8):
            ns = slice(n * 512, (n + 1) * 512)
            psq = ps_st.tile([128, 512], f32, tag="st")
            nc.tensor.matmul(psq, lhsT=wqt_sb[:, 0, :], rhs=src_sb[:, 0, ns],
                             start=True, stop=False)
            nc.tensor.matmul(psq, lhsT=wqt_sb[:, 1, :], rhs=src_sb[:, 1, ns],
                             start=False, stop=True)
            nc.vector.tensor_scalar_add(q_sb[:, ns], psq, bq_sb)
            psk = ps_st.tile([128, 512], f32, tag="st")
            nc.tensor.matmul(psk, lhsT=wkt_sb[:, 0, :], rhs=ctx_sb[:, 0, ns],
                             start=True, stop=False)
            nc.tensor.matmul(psk, lhsT=wkt_sb[:, 1, :], rhs=ctx_sb[:, 1, ns],
                             start=False, stop=True)
            nc.vector.tensor_scalar_add(k_sb[:, ns], psk, bk_sb)

        # v^T [j, c] per 128-row j chunk; bias added on DVE during the
        # PSUM->SBUF move (broadcast bv tile), not on the PE.
        for jc in range(32):
            js = slice(jc * 128, (jc + 1) * 128)
            psv = ps_st.tile([128, C], f32, tag="st")
            nc.tensor.matmul(psv, lhsT=ctx_sb[:, 0, js], rhs=wvt_sb[:, 0, :],
                             start=True, stop=False)
            nc.tensor.matmul(psv, lhsT=ctx_sb[:, 1, js], rhs=wvt_sb[:, 1, :],
                             start=False, stop=True)
            nc.vector.tensor_add(vT_sb[:, jc, 0:256], psv, bv_bc)

        # ---- attention, software-pipelined ----
        # S^T/exp for i-tile t+1 is emitted interleaved with the out-matmuls
        # of i-tile t, so the single S^T PSUM buffer never stalls the PE:
        # between two quads the PE always has out-matmul work, and the exp
        # of a quad runs on ACT in that shadow.
        def st_group(t, g):
            """S^T + exp for j chunks [g*ST_GROUP, (g+1)*ST_GROUP) of i-tile t."""
            isl = slice(t * 512, (t + 1) * 512)
            ps = ps_st.tile([128, 512 * ST_GROUP], f32, tag="st")
            for s in range(ST_GROUP):
                jc = g * ST_GROUP + s
                strip = jc % 4
                pb = slice(32 * strip, 32 * strip + 32)
                nc.tensor.matmul(
                    ps[:, s * 512:(s + 1) * 512],
                    lhsT=k_sb[pb, jc * 128:(jc + 1) * 128],
                    rhs=q_sb[pb, isl],
                    start=True, stop=True,
                    tile_position=(32 * strip, 0),
                )
            pt = p_pool.tile([128, 512 * ST_GROUP], bf16, tag="P")
            nc.scalar.activation(pt, ps, Exp, bias=ebias_sb)
            return pt

        def p_slice(p_tiles, jc, u):
            pt = p_tiles[jc // ST_GROUP]
            off = (jc % ST_GROUP) * 512 + u * 128
            return pt[:, off:off + 128]

        p_cur = [st_group(0, g) for g in range(N_GROUPS)]
        for t in range(8):  # i tiles of 512 query positions
            p_next = []
            for u in range(4):  # 128-row output chunks within the i tile
                gpu = N_GROUPS // 4  # groups to emit per u
                if t + 1 < 8:
                    for g in range(gpu * u, gpu * u + (gpu + 1) // 2):
                        p_next.append(st_group(t + 1, g))
                po = ps_out.tile([128, 257], f32, tag="po")
                for jc in range(16):
                    nc.tensor.matmul(po, lhsT=p_slice(p_cur, jc, u),
                                     rhs=vT_sb[:, jc, :],
                                     start=(jc == 0), stop=False)
                if t + 1 < 8:
                    for g in range(gpu * u + (gpu + 1) // 2, gpu * (u + 1)):
                        p_next.append(st_group(t + 1, g))
                for jc in range(16, 32):
                    nc.tensor.matmul(po, lhsT=p_slice(p_cur, jc, u),
                                     rhs=vT_sb[:, jc, :],
                                     start=False, stop=(jc == 31))
                rec = r_pool.tile([128, 1], f32, tag="rec")
                nc.vector.reciprocal(rec, po[:, 256:257])
                osb = o_pool.tile([128, 256], f32, tag="osb")
                nc.vector.tensor_scalar_mul(osb, po[:, 0:256], rec)
                row = t * 512 + u * 128
                nc.sync.dma_start(out=OUT[row:row + 128, :], in_=osb)
            p_cur = p_next

    nc.finalize()
    return nc


def _prep_shared(Wq, bq, Wk, bk, Wv, bv):
    fp16 = ml_dtypes.float16 if hasattr(ml_dtypes, "float16") else np.float16
    wqt = np.tile(np.ascontiguousarray(Wq.T), (1, 4)).astype(np.float16)
    wkt = np.tile(np.ascontiguousarray(Wk.T), (1, 4)).astype(np.float16)
    wvt = np.ascontiguousarray(Wv.T).astype(np.float16)
    bq_rep = np.tile(bq.astype(np.float32), 4)[:, None]
    bk_rep = np.tile(bk.astype(np.float32), 4)[:, None]
    bv_row = bv.astype(np.float32)[None, :]
    return {
        "wqt": wqt, "wkt": wkt, "wvt": wvt,
        "bq_rep": np.ascontiguousarray(bq_rep),
        "bk_rep": np.ascontiguousarray(bk_rep),
        "bv_row": np.ascontiguousarray(bv_row.astype(np.float32)),
    }


def kernel(x, y, Wq, bq, Wk, bk, Wv, bv):
    from concourse.bass_utils import run_bass_kernel_spmd

    if "nc" not in _cache:
        _cache["nc"] = _build_program()
    nc = _cache["nc"]

    shared = _prep_shared(Wq, bq, Wk, bk, Wv, bv)
    x2 = np.asarray(x, dtype=np.float32).reshape(B, C, HW)
    y2 = np.asarray(y, dtype=np.float32).reshape(B, C, HW)

    in_maps = []
    for core in range(N_CORES):
        d, b = divmod(core, B)
        src = x2[b] if d == 0 else y2[b]
        ctxm = y2[b] if d == 0 else x2[b]
        m = dict(shared)
        m["src"] = np.ascontiguousarray(src.astype(np.float16))
        m["ctx"] = np.ascontiguousarray(ctxm.astype(np.float16))
        in_maps.append(m)

    res = run_bass_kernel_spmd(nc, in_maps, list(range(N_CORES)))
    outs = [r["out_t"] for r in res.results]  # each [HW, C] fp32, transposed

    outx = np.stack([np.ascontiguousarray(outs[b].T).reshape(C, 64, 64)
                     for b in range(B)])
    outy = np.stack([np.ascontiguousarray(outs[B + b].T).reshape(C, 64, 64)
                     for b in range(B)])
    return (outx.astype(np.float32), outy.astype(np.float32))



# revision 11
# speedup vs baseline: 1.1086x; 1.1086x over previous
"""Bass/Trainium2 kernel for nn_CrossAttention_57964878627478.

Reference computation (per batch b, per direction):
    q = Wq @ src + bq            [32, 4096]   (src = x for dir 0, y for dir 1)
    k = Wk @ ctx + bk            [32, 4096]   (ctx = the other tensor)
    v = Wv @ ctx + bv            [256, 4096]
    attn = softmax_j(q^T k)      [4096, 4096]
    out  = v @ attn^T            [256, 4096]

Sharding: 8 independent (batch, direction) pairs -> one per NeuronCore.

Schedule (per core), tuned against the TimelineSim cost model where a
matmul costs output_free_size x pe_cycle and the PE p-state ramp clock
starts at the first PE instruction whose deps are satisfied:
  * Two dep-free "warmup" matmuls at t~0.3us start the ramp clock during
    the DMA dead time, so every real matmul runs at full pe_cycle.
  * DMA order: packed qk-weights+biases first, then src chunk 0 (2 halves),
    ctx chunks 0..7 with the packed v-weights early, then src 1..7. All
    weights/biases travel as one or two small [128, W] fp16 images.
  * Phase A: q0 projection; then per ctx chunk n: k(n), and (trailing one
    chunk) S^T(0) groups interleaved with v projections as PE filler so the
    ACT exp stream paces without idling the PE. Remaining v/q chunks drain
    right after. S^T = k^T q in [j, i] layout (contract-ready, K=32).
  * exp on ACT with constant bias shift (softmax is shift-invariant),
    bf16 P tiles. Steady state: baseline pipeline - out-matmuls of i-tile
    t interleave S^T/exp of tile t+1; ones column appended to v^T makes
    PSUM col 256 the softmax denominator; per-partition reciprocal + mul
    on DVE; out^T [4096, 256] fp32 to HBM, host transposes.
  * q/k in fp16, P/v in bf16 (P needs bf16 exponent range).
  * The last output chunk is split into two column halves so the final
    DVE-normalize + DMA tail chain is shorter.
"""

import os
import sys

if "/opt/trn_rl_repo" not in sys.path:
    sys.path.insert(0, "/opt/trn_rl_repo")

import numpy as np

C = 256
CQ = 32
HW = 4096
B = 4
N_CORES = 8
EXP_BIAS = -12.0

_cache = {}


def _build_program():
    from contextlib import ExitStack

    import concourse.bacc as bacc
    import concourse.mybir as mybir
    import concourse.tile as tile

    fp16 = mybir.dt.float16
    bf16 = mybir.dt.bfloat16
    f32 = mybir.dt.float32

    nc = bacc.Bacc(None, target_bir_lowering=False, debug=False)
    SRC = nc.dram_tensor("src", [C, HW], fp16, kind="ExternalInput")
    CTX = nc.dram_tensor("ctx", [C, HW], fp16, kind="ExternalInput")
    # wpk1: [wqT c0 | wqT c1 | wkT c0 | wkT c1 | bq | bk] = 130 cols
    WPK1 = nc.dram_tensor("wpk1", [128, 130], fp16, kind="ExternalInput")
    # wpk2: [wvT c0 (256) | wvT c1 (256) | bv broadcast (256)] = 768 cols
    WPK2 = nc.dram_tensor("wpk2", [128, 768], fp16, kind="ExternalInput")
    OUT = nc.dram_tensor("out_t", [HW, C], f32, kind="ExternalOutput")

    Exp = mybir.ActivationFunctionType.Exp

    JUNK_N = int(os.environ.get("K_JUNK_N", "2"))
    V_IN_LOOP = int(os.environ.get("K_V_IN_LOOP", "1"))  # v projs per st group
    N_GROUPS = 16  # S^T groups per i-tile (2 j-chunks each)

    with tile.TileContext(nc) as tc, ExitStack() as ctx:
        consts = ctx.enter_context(tc.tile_pool(name="consts", bufs=1))
        # PSUM budget (8 banks x 2KB): st 3 x [128,1024] f32 = 6 banks,
        # po 2 x [128,257] f32 = 2 banks.
        ps_st = ctx.enter_context(tc.tile_pool(name="ps_st", bufs=3, space="PSUM"))
        ps_out = ctx.enter_context(tc.tile_pool(name="ps_out", bufs=2, space="PSUM"))
        p_pool = ctx.enter_context(tc.tile_pool(name="p_pool", bufs=2 * N_GROUPS))
        o_pool = ctx.enter_context(tc.tile_pool(name="o_pool", bufs=4))
        r_pool = ctx.enter_context(tc.tile_pool(name="r_pool", bufs=4))

        src_sb = consts.tile([128, 2, HW], fp16, tag="src_sb")
        ctx_sb = consts.tile([128, 2, HW], fp16, tag="ctx_sb")
        wpk1_sb = consts.tile([128, 130], fp16, tag="wpk1_sb")
        wpk2_sb = consts.tile([128, 768], fp16, tag="wpk2_sb")
        q_sb = consts.tile([CQ, HW], fp16, tag="q_sb")
        k_sb = consts.tile([CQ, HW], fp16, tag="k_sb")
        vT_sb = consts.tile([128, 32, 257], bf16, tag="vT_sb")
        ebias_sb = consts.tile([128, 1], f32, tag="ebias_sb")
        junk_sb = consts.tile([128, 64], fp16, tag="junk_sb")

        # ---- PE ramp-clock warmup: dep-free matmuls at t~0 ----
        nc.vector.memset(junk_sb, 0.0)
        for _ in range(JUNK_N):
            pj = ps_st.tile([64, 64], f32, tag="st", name="pj")
            nc.tensor.matmul(pj, lhsT=junk_sb, rhs=junk_sb[:, 0:64],
                             start=True, stop=True)

        # ---- DMA stream (SP issue order == HWDGE order) ----
        src_r = SRC[:].rearrange("(c p) j -> p c j", p=128)
        ctx_r = CTX[:].rearrange("(c p) j -> p c j", p=128)
        nc.sync.dma_start(out=wpk1_sb, in_=WPK1[:])
        nc.sync.dma_start(out=src_sb[:, :, 0:256], in_=src_r[:, :, 0:256])
        nc.sync.dma_start(out=src_sb[:, :, 256:512], in_=src_r[:, :, 256:512])
        for n in range(8):
            ns = slice(n * 512, (n + 1) * 512)
            nc.sync.dma_start(out=ctx_sb[:, :, ns], in_=ctx_r[:, :, ns])
            if n == 1:
                nc.sync.dma_start(out=wpk2_sb, in_=WPK2[:])
        for n in range(1, 8):
            ns = slice(n * 512, (n + 1) * 512)
            nc.sync.dma_start(out=src_sb[:, :, ns], in_=src_r[:, :, ns])

        nc.vector.memset(ebias_sb, EXP_BIAS)
        # vT layout: col 0 = ones (softmax denominator), cols 1..256 = v
        nc.vector.memset(vT_sb[:, :, 0], 1.0)

        # f32 copies of the packed biases (DVE ALU wants f32 operands)
        bqk_sb = consts.tile([CQ, 2], f32, tag="bqk_sb")
        nc.vector.tensor_copy(bqk_sb, wpk1_sb[0:CQ, 128:130])
        bv_sb = consts.tile([128, 256], f32, tag="bv_sb")
        nc.vector.tensor_copy(bv_sb, wpk2_sb[:, 512:768])

        # ---- projection helpers ----
        def q_proj(t, half=None):
            ns = slice(t * 512, (t + 1) * 512)
            if half is not None:
                ns = slice(t * 512 + half * 256, t * 512 + (half + 1) * 256)
            w = ns.stop - ns.start
            psq = ps_st.tile([CQ, 512], f32, tag="st", name="psq")
            nc.tensor.matmul(psq[:, 0:w], lhsT=wpk1_sb[:, 0:32],
                             rhs=src_sb[:, 0, ns], start=True, stop=False)
            nc.tensor.matmul(psq[:, 0:w], lhsT=wpk1_sb[:, 32:64],
                             rhs=src_sb[:, 1, ns], start=False, stop=True)
            nc.vector.tensor_scalar_add(q_sb[:, ns], psq[:, 0:w],
                                        bqk_sb[:, 0:1])

        def k_proj(n):
            ns = slice(n * 512, (n + 1) * 512)
            psk = ps_st.tile([CQ, 512], f32, tag="st", name="psk")
            nc.tensor.matmul(psk, lhsT=wpk1_sb[:, 64:96],
                             rhs=ctx_sb[:, 0, ns], start=True, stop=False)
            nc.tensor.matmul(psk, lhsT=wpk1_sb[:, 96:128],
                             rhs=ctx_sb[:, 1, ns], start=False, stop=True)
            nc.vector.tensor_scalar_add(k_sb[:, ns], psk,
                                        bqk_sb[:, 1:2])

        def v_proj(jc):
            js = slice(jc * 128, (jc + 1) * 128)
            psv = ps_out.tile([128, 257], f32, tag="po", name="psv")
            nc.tensor.matmul(psv[:, 0:256], lhsT=ctx_sb[:, 0, js],
                             rhs=wpk2_sb[:, 0:256], start=True, stop=False)
            nc.tensor.matmul(psv[:, 0:256], lhsT=ctx_sb[:, 1, js],
                             rhs=wpk2_sb[:, 256:512], start=False, stop=True)
            nc.vector.tensor_add(vT_sb[:, jc, 1:257], psv[:, 0:256],
                                 bv_sb)

        def st_group(t, g):
            """S^T + exp for j chunks [2g, 2g+1] of i-tile t."""
            isl = slice(t * 512, (t + 1) * 512)
            ps = ps_st.tile([128, 1024], f32, tag="st", name="st")
            for s in range(2):
                jc = 2 * g + s
                nc.tensor.matmul(
                    ps[:, s * 512:(s + 1) * 512],
                    lhsT=k_sb[:, jc * 128:(jc + 1) * 128],
                    rhs=q_sb[:, isl],
                    start=True, stop=True,
                )
            pt = p_pool.tile([128, 1024], bf16, tag="P")
            nc.scalar.activation(pt, ps, Exp, bias=ebias_sb)
            return pt

        # ---- Phase A: q0, k + S^T(0) + v interleave ----
        q_proj(0, half=0)
        q_proj(0, half=1)
        p_cur = [None] * N_GROUPS
        vq = 0  # next v j-chunk to emit
        for n in range(8):
            k_proj(n)
            if n >= 1:
                g0 = 2 * (n - 1)
                for g in (g0, g0 + 1):
                    p_cur[g] = st_group(0, g)
                    for _ in range(V_IN_LOOP):
                        if vq < 4 * n:  # ctx chunk vq//4 must be loaded
                            v_proj(vq)
                            vq += 1
        for g in (14, 15):
            p_cur[g] = st_group(0, g)
            for _ in range(V_IN_LOOP):
                if vq < 32:
                    v_proj(vq)
                    vq += 1
        # remaining v and q chunks (src 1..7 landed by now)
        qt = 1
        while vq < 32 or qt < 8:
            if vq < 32:
                v_proj(vq)
                vq += 1
            if vq < 32:
                v_proj(vq)
                vq += 1
            if qt < 8:
                q_proj(qt)
                qt += 1

        # ---- attention steady state (i-tiles 0..7) ----
        def p_slice(p_tiles, jc, u):
            pt = p_tiles[jc // 2]
            off = (jc % 2) * 512 + u * 128
            return pt[:, off:off + 128]

        for t in range(8):
            p_next = [None] * N_GROUPS
            for u in range(4):
                row = t * 512 + u * 128
                if t == 7 and u == 3:
                    # split final chunk: [denom + cols 0:128] first, then
                    # cols [128:256], so the tail normalize+DMA chain is
                    # short (second half's DVE+DMA chain only)
                    po = ps_out.tile([128, 257], f32, tag="po", name="po")
                    for jc in range(32):
                        nc.tensor.matmul(po[:, 0:129],
                                         lhsT=p_slice(p_cur, jc, u),
                                         rhs=vT_sb[:, jc, 0:129],
                                         start=(jc == 0), stop=(jc == 31))
                    rec = r_pool.tile([128, 1], f32, tag="rec")
                    nc.vector.reciprocal(rec, po[:, 0:1])
                    osb = o_pool.tile([128, 256], f32, tag="osb")
                    nc.vector.tensor_scalar_mul(osb[:, 0:128], po[:, 1:129],
                                                rec)
                    nc.sync.dma_start(out=OUT[row:row + 128, 0:128],
                                      in_=osb[:, 0:128])
                    for jc in range(32):
                        nc.tensor.matmul(po[:, 129:257],
                                         lhsT=p_slice(p_cur, jc, u),
                                         rhs=vT_sb[:, jc, 129:257],
                                         start=(jc == 0), stop=(jc == 31))
                    nc.vector.tensor_scalar_mul(osb[:, 128:256],
                                                po[:, 129:257], rec)
                    nc.sync.dma_start(out=OUT[row:row + 128, 128:256],
                                      in_=osb[:, 128:256])
                    continue
                if t + 1 < 8:
                    for g in range(4 * u, 4 * u + 2):
                        p_next[g] = st_group(t + 1, g)
                po = ps_out.tile([128, 257], f32, tag="po", name="po")
                for jc in range(16):
                    nc.tensor.matmul(po, lhsT=p_slice(p_cur, jc, u),
                                     rhs=vT_sb[:, jc, :],
                                     start=(jc == 0), stop=False)
                if t + 1 < 8:
                    for g in range(4 * u + 2, 4 * u + 4):
                        p_next[g] = st_group(t + 1, g)
                for jc in range(16, 32):
                    nc.tensor.matmul(po, lhsT=p_slice(p_cur, jc, u),
                                     rhs=vT_sb[:, jc, :],
                                     start=False, stop=(jc == 31))
                rec = r_pool.tile([128, 1], f32, tag="rec")
                nc.vector.reciprocal(rec, po[:, 0:1])
                osb = o_pool.tile([128, 256], f32, tag="osb")
                nc.vector.tensor_scalar_mul(osb, po[:, 1:257], rec)
                nc.sync.dma_start(out=OUT[row:row + 128, :], in_=osb)
            p_cur = p_next

    nc.finalize()
    return nc


def _prep_shared(Wq, bq, Wk, bk, Wv, bv):
    wpk1 = np.zeros((128, 130), dtype=np.float16)
    wpk1[:, 0:32] = Wq.T[0:128].astype(np.float16)
    wpk1[:, 32:64] = Wq.T[128:256].astype(np.float16)
    wpk1[:, 64:96] = Wk.T[0:128].astype(np.float16)
    wpk1[:, 96:128] = Wk.T[128:256].astype(np.float16)
    wpk1[0:32, 128] = bq.astype(np.float16)
    wpk1[0:32, 129] = bk.astype(np.float16)
    wpk2 = np.zeros((128, 768), dtype=np.float16)
    wpk2[:, 0:256] = Wv.T[0:128].astype(np.float16)
    wpk2[:, 256:512] = Wv.T[128:256].astype(np.float16)
    wpk2[:, 512:768] = np.broadcast_to(bv.astype(np.float16), (128, 256))
    return {"wpk1": np.ascontiguousarray(wpk1),
            "wpk2": np.ascontiguousarray(wpk2)}


def kernel(x, y, Wq, bq, Wk, bk, Wv, bv):
    from concourse.bass_utils import run_bass_kernel_spmd

    if "nc" not in _cache:
        _cache["nc"] = _build_program()
    nc = _cache["nc"]

    shared = _prep_shared(Wq, bq, Wk, bk, Wv, bv)
    x2 = np.asarray(x, dtype=np.float32).reshape(B, C, HW)
    y2 = np.asarray(y, dtype=np.float32).reshape(B, C, HW)

    in_maps = []
    for core in range(N_CORES):
        d, b = divmod(core, B)
        src = x2[b] if d == 0 else y2[b]
        ctxm = y2[b] if d == 0 else x2[b]
        m = dict(shared)
        m["src"] = np.ascontiguousarray(src.astype(np.float16))
        m["ctx"] = np.ascontiguousarray(ctxm.astype(np.float16))
        in_maps.append(m)

    res = run_bass_kernel_spmd(nc, in_maps, list(range(N_CORES)))
    outs = [r["out_t"] for r in res.results]  # each [HW, C] fp32, transposed

    outx = np.stack([np.ascontiguousarray(outs[b].T).reshape(C, 64, 64)
                     for b in range(B)])
    outy = np.stack([np.ascontiguousarray(outs[B + b].T).reshape(C, 64, 64)
                     for b in range(B)])
    return (outx.astype(np.float32), outy.astype(np.float32))


# revision 34
# speedup vs baseline: 1.1220x; 1.0121x over previous
"""Bass/Trainium2 kernel for nn_CrossAttention_57964878627478.

Reference computation (per batch b, per direction):
    q = Wq @ src + bq            [32, 4096]   (src = x for dir 0, y for dir 1)
    k = Wk @ ctx + bk            [32, 4096]   (ctx = the other tensor)
    v = Wv @ ctx + bv            [256, 4096]
    attn = softmax_j(q^T k)      [4096, 4096]
    out  = v @ attn^T            [256, 4096]

Sharding: 8 independent (batch, direction) pairs -> one per NeuronCore.

Schedule (per core), tuned against the TimelineSim cost model where a
matmul costs output_free_size x pe_cycle and the PE p-state ramp clock
starts at the first PE instruction whose deps are satisfied:
  * Two dep-free "warmup" matmuls at t~0.3us start the ramp clock during
    the DMA dead time, so every real matmul runs at full pe_cycle.
  * DMA order: packed qk-weights+biases first, then src chunk 0 (2 halves),
    ctx chunks 0..7 with the packed v-weights early, then src 1..7. All
    weights/biases travel as one or two small [128, W] fp16 images.
  * Phase A: q0 projection; then per ctx chunk n: k(n), and (trailing one
    chunk) S^T(0) groups interleaved with v projections as PE filler so the
    ACT exp stream paces without idling the PE. Remaining v/q chunks drain
    right after. S^T = k^T q in [j, i] layout (contract-ready, K=32).
  * exp on ACT with constant bias shift (softmax is shift-invariant),
    bf16 P tiles. Steady state: baseline pipeline - out-matmuls of i-tile
    t interleave S^T/exp of tile t+1; ones column appended to v^T makes
    PSUM col 256 the softmax denominator; per-partition reciprocal + mul
    on DVE; out^T [4096, 256] fp32 to HBM, host transposes.
  * q/k in fp16, P/v in bf16 (P needs bf16 exponent range).
  * The last output chunk is split into two column halves so the final
    DVE-normalize + DMA tail chain is shorter.
"""

import os
import sys

if "/opt/trn_rl_repo" not in sys.path:
    sys.path.insert(0, "/opt/trn_rl_repo")

import numpy as np

C = 256
CQ = 32
HW = 4096
B = 4
N_CORES = 8
EXP_BIAS = -12.0

_cache = {}


def _build_program():
    from contextlib import ExitStack

    import concourse.bacc as bacc
    import concourse.mybir as mybir
    import concourse.tile as tile

    fp16 = mybir.dt.float16
    bf16 = mybir.dt.bfloat16
    f32 = mybir.dt.float32

    nc = bacc.Bacc(None, target_bir_lowering=False, debug=False)
    SRC = nc.dram_tensor("src", [C, HW], fp16, kind="ExternalInput")
    CTX = nc.dram_tensor("ctx", [C, HW], fp16, kind="ExternalInput")
    # wpk1: [wqT c0 | wqT c1 | wkT c0 | wkT c1 | bq | bk | src cols 0:512
    # as [c, j] image (2*512)] = 1154 cols -- src chunk 0 rides along so the
    # first q-projection needs just one DMA round-trip
    WPK1 = nc.dram_tensor("wpk1", [128, 1154], fp16, kind="ExternalInput")
    # wpk2: [wvT c0 (256) | wvT c1 (256) | bv broadcast (256)] = 768 cols
    WPK2 = nc.dram_tensor("wpk2", [128, 768], fp16, kind="ExternalInput")
    OUT = nc.dram_tensor("out_t", [HW, C], f32, kind="ExternalOutput")

    Exp = mybir.ActivationFunctionType.Exp

    JUNK_N = int(os.environ.get("K_JUNK_N", "2"))
    V_IN_LOOP = int(os.environ.get("K_V_IN_LOOP", "1"))  # v projs per st group
    N_GROUPS = 16  # S^T groups per i-tile (2 j-chunks each)

    with tile.TileContext(nc) as tc, ExitStack() as ctx:
        consts = ctx.enter_context(tc.tile_pool(name="consts", bufs=1))
        # PSUM budget (8 banks x 2KB): st 3 x [128,1024] f32 = 6 banks,
        # po 2 x [128,257] f32 = 2 banks.
        ps_st = ctx.enter_context(tc.tile_pool(name="ps_st", bufs=3, space="PSUM"))
        ps_out = ctx.enter_context(tc.tile_pool(name="ps_out", bufs=2, space="PSUM"))
        p_pool = ctx.enter_context(tc.tile_pool(name="p_pool", bufs=2 * N_GROUPS))
        o_pool = ctx.enter_context(tc.tile_pool(name="o_pool", bufs=4))
        r_pool = ctx.enter_context(tc.tile_pool(name="r_pool", bufs=4))

        src_sb = consts.tile([128, 2, HW], fp16, tag="src_sb")
        ctx_sb = consts.tile([128, 2, HW], fp16, tag="ctx_sb")
        wpk1_sb = consts.tile([128, 1154], fp16, tag="wpk1_sb")
        wpk2_sb = consts.tile([128, 768], fp16, tag="wpk2_sb")
        q_sb = consts.tile([CQ, HW], fp16, tag="q_sb")
        k_sb = consts.tile([CQ, HW], fp16, tag="k_sb")
        vT_sb = consts.tile([128, 32, 257], bf16, tag="vT_sb")
        ebias_sb = consts.tile([128, 1], f32, tag="ebias_sb")
        junk_sb = consts.tile([128, 64], fp16, tag="junk_sb")
        junkf_sb = consts.tile([128, 512], fp16, tag="junkf_sb")

        # ---- PE ramp-clock warmup: dep-free matmuls at t~0 ----
        nc.vector.memset(junk_sb, 0.0)
        for _ in range(JUNK_N):
            pj = ps_st.tile([64, 64], f32, tag="st", name="pj")
            nc.tensor.matmul(pj, lhsT=junk_sb, rhs=junk_sb,
                             start=True, stop=True)
        nc.vector.memset(junkf_sb, 0.0)

        # ---- DMA stream (SP issue order == HWDGE order) ----
        src_r = SRC[:].rearrange("(c p) j -> p c j", p=128)
        ctx_r = CTX[:].rearrange("(c p) j -> p c j", p=128)
        nc.sync.dma_start(out=wpk1_sb, in_=WPK1[:])
        for n in range(8):
            ns = slice(n * 512, (n + 1) * 512)
            nc.sync.dma_start(out=ctx_sb[:, :, ns], in_=ctx_r[:, :, ns])
            if n == 1:
                nc.sync.dma_start(out=wpk2_sb, in_=WPK2[:])
        for n in range(1, 8):
            ns = slice(n * 512, (n + 1) * 512)
            nc.sync.dma_start(out=src_sb[:, :, ns], in_=src_r[:, :, ns])

        nc.vector.memset(ebias_sb, EXP_BIAS)
        # vT layout: col 0 = ones (softmax denominator), cols 1..256 = v
        nc.vector.memset(vT_sb[:, :, 0], 1.0)

        # f32 copies of the packed biases (DVE ALU wants f32 operands)
        bqk_sb = consts.tile([CQ, 2], f32, tag="bqk_sb")
        nc.vector.tensor_copy(bqk_sb, wpk1_sb[0:CQ, 128:130])
        bv_sb = consts.tile([128, 256], f32, tag="bv_sb")
        nc.vector.tensor_copy(bv_sb, wpk2_sb[:, 512:768])

        # ---- projection helpers ----
        def q_proj(t):
            ns = slice(t * 512, (t + 1) * 512)
            # src chunk 0 rides in wpk1 (cols 130:642 c-half0, 642:1154 c1)
            rhs0 = wpk1_sb[:, 130:642] if t == 0 else src_sb[:, 0, ns]
            rhs1 = wpk1_sb[:, 642:1154] if t == 0 else src_sb[:, 1, ns]
            psq = ps_st.tile([CQ, 512], f32, tag="st", name="psq")
            nc.tensor.matmul(psq, lhsT=wpk1_sb[:, 0:32],
                             rhs=rhs0, start=True, stop=False)
            nc.tensor.matmul(psq, lhsT=wpk1_sb[:, 32:64],
                             rhs=rhs1, start=False, stop=True)
            nc.vector.tensor_scalar_add(q_sb[:, ns], psq, bqk_sb[:, 0:1])

        def k_proj(n):
            ns = slice(n * 512, (n + 1) * 512)
            psk = ps_st.tile([CQ, 512], f32, tag="st", name="psk")
            nc.tensor.matmul(psk, lhsT=wpk1_sb[:, 64:96],
                             rhs=ctx_sb[:, 0, ns], start=True, stop=False)
            nc.tensor.matmul(psk, lhsT=wpk1_sb[:, 96:128],
                             rhs=ctx_sb[:, 1, ns], start=False, stop=True)
            if os.environ.get("K_KSPLIT", "0") == "1":
                # halved bias-adds: first 256 cols available to S^T sooner
                nc.vector.tensor_scalar_add(k_sb[:, ns.start:ns.start + 256],
                                            psk[:, 0:256], bqk_sb[:, 1:2])
                nc.vector.tensor_scalar_add(k_sb[:, ns.start + 256:ns.stop],
                                            psk[:, 256:512], bqk_sb[:, 1:2])
            else:
                nc.vector.tensor_scalar_add(k_sb[:, ns], psk, bqk_sb[:, 1:2])

        def v_proj(jc):
            js = slice(jc * 128, (jc + 1) * 128)
            psv = ps_out.tile([128, 257], f32, tag="po", name="psv")
            nc.tensor.matmul(psv[:, 0:256], lhsT=ctx_sb[:, 0, js],
                             rhs=wpk2_sb[:, 0:256], start=True, stop=False)
            nc.tensor.matmul(psv[:, 0:256], lhsT=ctx_sb[:, 1, js],
                             rhs=wpk2_sb[:, 256:512], start=False, stop=True)
            nc.vector.tensor_add(vT_sb[:, jc, 1:257], psv[:, 0:256],
                                 bv_sb)

        def st_group(t, g):
            """S^T + exp for j chunks [2g, 2g+1] of i-tile t."""
            isl = slice(t * 512, (t + 1) * 512)
            ps = ps_st.tile([128, 1024], f32, tag="st", name="st")
            for s in range(2):
                jc = 2 * g + s
                nc.tensor.matmul(
                    ps[:, s * 512:(s + 1) * 512],
                    lhsT=k_sb[:, jc * 128:(jc + 1) * 128],
                    rhs=q_sb[:, isl],
                    start=True, stop=True,
                )
            pt = p_pool.tile([128, 1024], bf16, tag="P")
            nc.scalar.activation(pt, ps, Exp, bias=ebias_sb)
            return pt

        # ---- Phase A: q0, k + S^T(0) + v interleave ----
        q_proj(0)
        del src_r  # src chunk 0 never loaded into src_sb (rides in wpk1)
        # junk filler: PE would otherwise idle on ctx0/ctx1 DMA latency
        JUNK_FILL = int(os.environ.get("K_JUNK_FILL", "1"))
        for _ in range(JUNK_FILL):
            pj = ps_st.tile([64, 512], f32, tag="st", name="pjf")
            nc.tensor.matmul(pj, lhsT=junk_sb, rhs=junkf_sb,
                             start=True, stop=True)
        p_cur = [None] * N_GROUPS
        vq = 0  # next v j-chunk to emit
        for n in range(8):
            k_proj(n)
            if n >= 1:
                g0 = 2 * (n - 1)
                for gi, g in enumerate((g0, g0 + 1)):
                    p_cur[g] = st_group(0, g)
                    # 3 v-projections per block: fills the ACT-paced
                    # (2 exps = 2076ns) block without overloading DVE
                    for _ in range(V_IN_LOOP + (1 - gi)):
                        if vq < 4 * n:  # ctx chunk vq//4 must be loaded
                            v_proj(vq)
                            vq += 1
        for g in (14, 15):
            p_cur[g] = st_group(0, g)
            for _ in range(V_IN_LOOP):
                if vq < 32:
                    v_proj(vq)
                    vq += 1
        # remaining v chunks; q1 (q for t>=2 is produced in the t-loop)
        while vq < 32:
            v_proj(vq)
            vq += 1
        q_proj(1)

        # ---- attention steady state (i-tiles 0..7) ----
        def p_slice(p_tiles, jc, u):
            pt = p_tiles[jc // 2]
            off = (jc % 2) * 512 + u * 128
            return pt[:, off:off + 128]

        for t in range(8):
            p_next = [None] * N_GROUPS
            if t + 2 < 8:
                q_proj(t + 2)  # q for i-tile t+2, used by st_group(t+1=..)
            for u in range(4):
                row = t * 512 + u * 128
                if t == 7 and u == 3:
                    # split final chunk: [denom + cols 0:192] first, then
                    # cols [192:256] in a separate PSUM tile, so the tail
                    # normalize+DMA chain is just the short second half
                    po = ps_out.tile([128, 257], f32, tag="po", name="po")
                    for jc in range(32):
                        nc.tensor.matmul(po[:, 0:193],
                                         lhsT=p_slice(p_cur, jc, u),
                                         rhs=vT_sb[:, jc, 0:193],
                                         start=(jc == 0), stop=(jc == 31))
                    rec = r_pool.tile([128, 1], f32, tag="rec")
                    nc.vector.reciprocal(rec, po[:, 0:1])
                    osb = o_pool.tile([128, 256], f32, tag="osb")
                    nc.vector.tensor_scalar_mul(osb[:, 0:192], po[:, 1:193],
                                                rec)
                    nc.sync.dma_start(out=OUT[row:row + 128, 0:192],
                                      in_=osb[:, 0:192])
                    po2 = ps_out.tile([128, 257], f32, tag="po", name="po2")
                    for jc in range(32):
                        nc.tensor.matmul(po2[:, 0:64],
                                         lhsT=p_slice(p_cur, jc, u),
                                         rhs=vT_sb[:, jc, 193:257],
                                         start=(jc == 0), stop=(jc == 31))
                    nc.vector.tensor_scalar_mul(osb[:, 192:256],
                                                po2[:, 0:64], rec)
                    nc.sync.dma_start(out=OUT[row:row + 128, 192:256],
                                      in_=osb[:, 192:256])
                    continue
                if t + 1 < 8:
                    for g in range(4 * u, 4 * u + 2):
                        p_next[g] = st_group(t + 1, g)
                po = ps_out.tile([128, 257], f32, tag="po", name="po")
                for jc in range(16):
                    nc.tensor.matmul(po, lhsT=p_slice(p_cur, jc, u),
                                     rhs=vT_sb[:, jc, :],
                                     start=(jc == 0), stop=False)
                if t + 1 < 8:
                    for g in range(4 * u + 2, 4 * u + 4):
                        p_next[g] = st_group(t + 1, g)
                for jc in range(16, 32):
                    nc.tensor.matmul(po, lhsT=p_slice(p_cur, jc, u),
                                     rhs=vT_sb[:, jc, :],
                                     start=False, stop=(jc == 31))
                rec = r_pool.tile([128, 1], f32, tag="rec")
                nc.vector.reciprocal(rec, po[:, 0:1])
                osb = o_pool.tile([128, 256], f32, tag="osb")
                nc.vector.tensor_scalar_mul(osb, po[:, 1:257], rec)
                nc.sync.dma_start(out=OUT[row:row + 128, :], in_=osb)
            p_cur = p_next

    nc.finalize()
    return nc


def _prep_shared(Wq, bq, Wk, bk, Wv, bv):
    wpk1 = np.zeros((128, 130), dtype=np.float16)
    wpk1[:, 0:32] = Wq.T[0:128].astype(np.float16)
    wpk1[:, 32:64] = Wq.T[128:256].astype(np.float16)
    wpk1[:, 64:96] = Wk.T[0:128].astype(np.float16)
    wpk1[:, 96:128] = Wk.T[128:256].astype(np.float16)
    wpk1[0:32, 128] = bq.astype(np.float16)
    wpk1[0:32, 129] = bk.astype(np.float16)
    wpk2 = np.zeros((128, 768), dtype=np.float16)
    wpk2[:, 0:256] = Wv.T[0:128].astype(np.float16)
    wpk2[:, 256:512] = Wv.T[128:256].astype(np.float16)
    wpk2[:, 512:768] = np.broadcast_to(bv.astype(np.float16), (128, 256))
    return {"wpk1_base": np.ascontiguousarray(wpk1),
            "wpk2": np.ascontiguousarray(wpk2)}


def kernel(x, y, Wq, bq, Wk, bk, Wv, bv):
    from concourse.bass_utils import run_bass_kernel_spmd

    if "nc" not in _cache:
        _cache["nc"] = _build_program()
    nc = _cache["nc"]

    shared = _prep_shared(Wq, bq, Wk, bk, Wv, bv)
    x2 = np.asarray(x, dtype=np.float32).reshape(B, C, HW)
    y2 = np.asarray(y, dtype=np.float32).reshape(B, C, HW)

    in_maps = []
    for core in range(N_CORES):
        d, b = divmod(core, B)
        src = x2[b] if d == 0 else y2[b]
        ctxm = y2[b] if d == 0 else x2[b]
        src16 = src.astype(np.float16)
        # per-core wpk1 = [shared weights/biases | src chunk 0 [c, 0:512]]
        wpk1 = np.concatenate(
            [shared["wpk1_base"], src16[0:128, 0:512], src16[128:256, 0:512]],
            axis=1)
        m = {"wpk2": shared["wpk2"],
             "wpk1": np.ascontiguousarray(wpk1),
             "src": np.ascontiguousarray(src16),
             "ctx": np.ascontiguousarray(ctxm.astype(np.float16))}
        in_maps.append(m)

    res = run_bass_kernel_spmd(nc, in_maps, list(range(N_CORES)))
    outs = [r["out_t"] for r in res.results]  # each [HW, C] fp32, transposed

    outx = np.stack([np.ascontiguousarray(outs[b].T).reshape(C, 64, 64)
                     for b in range(B)])
    outy = np.stack([np.ascontiguousarray(outs[B + b].T).reshape(C, 64, 64)
                     for b in range(B)])
    return (outx.astype(np.float32), outy.astype(np.float32))
